# revision 1
# baseline (speedup 1.0000x reference)
"""MoE layer (8 experts, top-2) on 8 Trainium2 NeuronCores.

Strategy: expert parallelism with host-side dispatch.
  - Host: gate logits (tiny matmul), top-2 + softmax, token->expert dispatch
    with capacity padding, weight/activation pre-layout + cast to bf16.
  - Core e: y_e = relu(x_e @ w1[e].T) @ w2[e].T * gate_scale  (bf16 matmuls,
    fp32 PSUM accumulation), tokens dispatched to expert e only.
  - Host: scatter-add the two expert contributions per token (fp32, exact).

Kernel dataflow (per core):
  layer 1 keeps D on partitions: hT[f, c] = relu(sum_d w1T[d, f] xT[d, c]),
  layer 2 keeps tokens on partitions: y[c, d] = s[c] sum_f hT[f, c] w2T[f, d].
  Both layers are back-to-back matmul streams on TensorE; relu + gate scaling
  ride on ScalarE. Input DMA is staged: the first w1 chunk and first x block
  are split across queues and start immediately; the remaining weight DMAs
  are dependency-delayed behind early compute so they don't steal bandwidth
  from the critical path.
"""

import os

os.environ.setdefault("BASS_NEVER_TRACE", "1")

import numpy as np
import ml_dtypes

D_MODEL = 1024
D_FF = 4096
NUM_EXPERTS = 8
TOP_K = 2
P = 128
C_BLK = 512
WCH = 4  # fc / kf chunk size for weight DMA staging

BF16 = ml_dtypes.bfloat16

_NC_CACHE: dict[int, object] = {}


def _block_widths(C: int) -> list[int]:
    """Split C into ceil(C/512) blocks, multiples of 128, each >=256 when
    possible (N>=256 keeps matmul issue stream-bound, not LDWEIGHTS-bound)."""
    n_blocks = -(-C // C_BLK)
    widths, rem = [], C
    for i in range(n_blocks):
        w = max(256, min(C_BLK, rem - 256 * (n_blocks - 1 - i)))
        widths.append(w)
        rem -= w
    if sum(widths) != C or any(x % 128 or x <= 0 or x > C_BLK for x in widths):
        widths, rem = [], C
        while rem:
            widths.append(min(C_BLK, rem))
            rem -= widths[-1]
    # smallest block first: PE starts sooner (less x/w1 data on the critical
    # path) and warms up on the cheap block while the weight DMAs stream
    return sorted(widths)


def build_moe_nc(C: int):
    """Bass/Tile program for one expert shard with token capacity C.

    DRAM inputs (per core):
      xs  [128, KD, C]        bf16   x_e.T striped: xs[p, k, c] = x_e[c, k*128+p]
      w1s [128, KF, KD, 128]  bf16   w1s[p, fc, k, j] = w1[e][fc*128+j, k*128+p]
      w2s [128, KF, D]        bf16   w2s[p, kf, d]    = w2[e][d, kf*128+p]
      ss  [128, C//128]       f32    ss[p, j] = gate_scale[j*128+p]
    DRAM output:
      y   [C, D] f32          y[c] = gate_scale[c] * relu(x_e[c] @ w1.T) @ w2.T
    """
    import concourse.mybir as mybir
    import concourse.tile as tile
    from concourse import bacc
    from concourse.tile import add_dep_helper

    D, F = D_MODEL, D_FF
    KD, KF = D // P, F // P  # 8, 32
    bf16, f32 = mybir.dt.bfloat16, mybir.dt.float32
    Relu = mybir.ActivationFunctionType.Relu
    assert C % P == 0

    nc = bacc.Bacc("TRN2", target_bir_lowering=False, debug=False)
    xs = nc.dram_tensor("xs", [P, KD, C], bf16, kind="ExternalInput")
    w1s = nc.dram_tensor("w1s", [P, KF, KD, P], bf16, kind="ExternalInput")
    w2s = nc.dram_tensor("w2s", [P, KF, D], bf16, kind="ExternalInput")
    ss = nc.dram_tensor("ss", [P, C // P], f32, kind="ExternalInput")
    y = nc.dram_tensor("y", [C, D], f32, kind="ExternalOutput")

    blocks = []
    off = 0
    for w in _block_widths(C):
        blocks.append((off, w))
        off += w

    with tile.TileContext(nc) as tc:
        with (
            tc.tile_pool(name="wpool", bufs=1) as wpool,
            tc.tile_pool(name="xpool", bufs=2) as xpool,
            tc.tile_pool(name="hpool", bufs=1) as hpool,
            tc.tile_pool(name="ypool", bufs=3) as ypool,
            tc.tile_pool(name="phpool", bufs=3, space="PSUM") as phpool,
            tc.tile_pool(name="pypool", bufs=4, space="PSUM") as pypool,
        ):
            s_sb = wpool.tile([P, C // P], f32)
            nc.sync.dma_start(s_sb[:], ss[:])
            # weights as separate chunk tiles; chunk 0 of w1 split across
            # queues (critical path), the rest dependency-delayed below
            w1_ch, w2_ch, w1_dma, w2_dma = [], [], [], []
            for fc0 in range(0, KF, WCH):
                t = wpool.tile([P, WCH, KD, P], bf16, tag=f"w1_{fc0}")
                subs = [
                    nc.sync.dma_start(t[:, j : j + 1], w1s[:, fc0 + j : fc0 + j + 1])
                    for j in range(WCH)
                ]
                w1_ch.append(t)
                w1_dma.append(subs)
            for k0 in range(0, KF, WCH):
                t = wpool.tile([P, WCH, D], bf16, tag=f"w2_{k0}")
                subs = [
                    nc.sync.dma_start(t[:, j : j + 2], w2s[:, k0 + j : k0 + j + 2])
                    for j in range(0, WCH, 2)
                ]
                w2_ch.append(t)
                w2_dma.append(subs)

            def w1_ap(fc, k):
                return w1_ch[fc // WCH][:, fc % WCH, k]

            def w2_ap(k, nsl):
                return w2_ch[k // WCH][:, k % WCH, nsl]

            first_mm = None
            relu_gate = None
            for bi, (off, w) in enumerate(blocks):
                xt = xpool.tile([P, KD, C_BLK], bf16, tag="xt")
                if bi == 0:
                    for k0 in range(0, KD, 2):
                        nc.sync.dma_start(
                            xt[:, k0 : k0 + 2, :w], xs[:, k0 : k0 + 2, off : off + w]
                        )
                else:
                    xd = nc.sync.dma_start(xt[:, :, :w], xs[:, :, off : off + w])
                    if bi == 1 and relu_gate is not None:
                        add_dep_helper(xd.ins, relu_gate.ins, reason="stage xt1")
                hT = hpool.tile([P, KF, C_BLK], bf16, tag="hT")
                # layer 1
                for fc in range(KF):
                    ph = phpool.tile([P, C_BLK], f32, tag="ph")
                    for k in range(KD):
                        mm = nc.tensor.matmul(
                            ph[:, :w],
                            lhsT=w1_ap(fc, k),
                            rhs=xt[:, k, :w],
                            start=(k == 0),
                            stop=(k == KD - 1),
                        )
                        if first_mm is None:
                            first_mm = mm
                            for subs in w1_dma[1:]:
                                for d in subs:
                                    add_dep_helper(d.ins, first_mm.ins, reason="stage w1")
                    # relu on DVE (VectorE) — ACT handles only the gate scaling,
                    # keeping both engines far under the TensorE span
                    act = nc.vector.tensor_scalar_max(hT[:, fc, :w], ph[:, :w], 0.0)
                    if bi == 0 and fc == 8 and relu_gate is None:
                        relu_gate = act
                        for subs in w2_dma:
                            for d in subs:
                                add_dep_helper(d.ins, relu_gate.ins, reason="stage w2")
                # layer 2
                for c0 in range(0, w, P):
                    ys = ypool.tile([P, D], f32, tag="ys")
                    j = (off + c0) // P
                    for ns in range(D // 512):
                        py = pypool.tile([P, 512], f32, tag="py")
                        for k in range(KF):
                            nc.tensor.matmul(
                                py,
                                lhsT=hT[:, k, c0 : c0 + P],
                                rhs=w2_ap(k, slice(ns * 512, (ns + 1) * 512)),
                                start=(k == 0),
                                stop=(k == KF - 1),
                            )
                        nc.scalar.mul(ys[:, ns * 512 : (ns + 1) * 512], py, s_sb[:, j : j + 1])
                    nc.sync.dma_start(y[off + c0 : off + c0 + P, :], ys[:])

    nc.compile()
    return nc


def route_tokens(xf: np.ndarray, gate_w: np.ndarray):
    """Top-2 routing, replicating jax.lax.top_k tie-breaking (lowest index)."""
    logits = xf @ gate_w.astype(np.float32).T  # [T, E]
    top2 = np.argsort(-logits, axis=-1, kind="stable")[:, :TOP_K]
    tv = np.take_along_axis(logits, top2, axis=-1)
    tv = tv - tv.max(axis=-1, keepdims=True)
    ex = np.exp(tv)
    gates = ex / ex.sum(axis=-1, keepdims=True)
    rows, weights = [], []
    for e in range(NUM_EXPERTS):
        r, kpos = np.nonzero(top2 == e)
        rows.append(r)
        weights.append(gates[r, kpos].astype(np.float32))
    return rows, weights


def make_expert_inputs(xf, w1, w2, rows, weights, C):
    """Per-core input arrays in the DRAM layouts build_moe_nc expects."""
    KD, KF = D_MODEL // P, D_FF // P
    in_maps = []
    for e in range(NUM_EXPERTS):
        cnt = len(rows[e])
        X = np.zeros((C, D_MODEL), BF16)
        X[:cnt] = xf[rows[e]].astype(BF16)
        # [C, D] -> [p, k, c]
        xs = np.ascontiguousarray(X.T.reshape(KD, P, C).transpose(1, 0, 2))
        W1 = w1[e].astype(BF16)  # [F, D]
        w1s = np.ascontiguousarray(
            W1.reshape(KF, P, KD, P).transpose(3, 0, 2, 1)
        )  # [p, fc, k, fcol]
        W2 = w2[e].astype(BF16)  # [D, F]
        w2s = np.ascontiguousarray(W2.T.reshape(KF, P, D_MODEL).transpose(1, 0, 2))
        s = np.zeros((C,), np.float32)
        s[:cnt] = weights[e]
        ss = np.ascontiguousarray(s.reshape(C // P, P).T)
        in_maps.append({"xs": xs, "w1s": w1s, "w2s": w2s, "ss": ss})
    return in_maps


def kernel(x, gate_w, w1, w2):
    from concourse.bass_utils import run_bass_kernel_spmd

    x = np.asarray(x)
    gate_w = np.asarray(gate_w)
    w1 = np.asarray(w1)
    w2 = np.asarray(w2)
    B, S, D = x.shape

    xf = x.reshape(-1, D).astype(np.float32)
    rows, weights = route_tokens(xf, gate_w)
    counts = [len(r) for r in rows]
    C = max(C_BLK, -(-max(counts) // P) * P)

    nc = _NC_CACHE.get(C)
    if nc is None:
        nc = _NC_CACHE[C] = build_moe_nc(C)
    in_maps = make_expert_inputs(xf, w1, w2, rows, weights, C)
    res = run_bass_kernel_spmd(nc, in_maps, core_ids=list(range(NUM_EXPERTS)))

    out = np.zeros((B * S, D), np.float32)
    for e in range(NUM_EXPERTS):
        out[rows[e]] += res.results[e]["y"][: counts[e]]
    return out.reshape(B, S, D)



# revision 3
# speedup vs baseline: 1.2909x; 1.2909x over previous
"""MoE layer (8 experts, top-2) on 8 Trainium2 NeuronCores.

Expert parallelism with host-side dispatch, fp8 DoubleRow matmuls with full
error compensation:
  - Host: gate logits, top-2 + softmax, token->expert dispatch. Gate scales
    are folded into x (relu is positive-homogeneous), so the device computes
    plain y_e = relu(x_e @ w1.T) @ w2.T on pre-scaled tokens.
  - Every tensor T is split as T = T0 + T1 with both halves fp8 e4m3
    (T0 = fp8(T), T1 = fp8(T - T0)); weight residuals are pre-scaled by 2^6
    so they stay clear of the fp8 subnormal floor. Each 128x128-pair matmul
    then runs in MatmulPerfMode.DoubleRow (two stationary/moving k-tile pairs
    summed per instruction at 0.5 cycles/row — 4x bf16 throughput):
      psum_main = w0.T@x0 + w0.T@x1       (exact scale)
      psum_corr = (w1*2^6).T@x0           (scale 2^6)
      result    = psum_main + 2^-6 * psum_corr
    dropping only the w1.T@x1 term (~1e-3 relative). End-to-end error matches
    bf16 (~3e-3) at 1.5x the fp8 peak rate.
  - Both layers keep tokens on the moving free dim (any block width, no
    128-token padding): L1 h[f,c], L2 yT[d,c]; host transposes y back.
  - DMA issue order is hand-scheduled so the head of the FIFO is always the
    next thing compute needs: x(block0) + first w1 chunks, then interleaved
    w1/w2 chunk streams sized so layer-2 weights land before block0's L1 ends.
"""

import os

os.environ.setdefault("BASS_NEVER_TRACE", "1")

import numpy as np
import ml_dtypes

D_MODEL = 1024
D_FF = 4096
NUM_EXPERTS = 8
TOP_K = 2
P = 128
KD = D_MODEL // P  # 8
KF = D_FF // P  # 32
C_BLK = 512
WCH = 4  # fc chunk size for w1 DMA staging
SC = 64.0  # residual pre-scale (2^6)

F8 = ml_dtypes.float8_e4m3

_NC_CACHE: dict[int, object] = {}


def _block_widths(C: int) -> list[int]:
    """First block 512 (buys time for the w2 DMA stream before L2 starts),
    the rest equal-ish. Every block must be >=342 wide so the PE engine time
    per matmul (0.5 cyc/row) stays above the 71 ns sequencer issue cost."""
    if C <= C_BLK:
        return [C]
    nb = -(-C // C_BLK)
    rest = C - C_BLK
    if nb == 1:
        return [C]
    base = rest // (nb - 1)
    remn = rest % (nb - 1)
    widths = [C_BLK] + [base + (1 if i < remn else 0) for i in range(nb - 1)]
    assert sum(widths) == C
    return widths


def capacity(max_count: int) -> int:
    # exact max count, but keep blocks wide enough to stay engine-bound
    C = max(max_count, 2 * 342)
    return C


def build_moe_nc(C: int):
    """Bass/Tile program for one expert shard with token capacity C.

    DRAM inputs (per core), all fp8 e4m3 unless noted:
      xs0 [128, KD, C]        xs0[p,k,c] = fp8(g_c * x_c)[k*128+p]
      xs1 [128, KD, C]        fp8 residual of the above
      w1a [128, KF, KD, 128]  w1a[p,fc,k,j] = fp8(w1[fc*128+j, k*128+p])
      w1b [128, KF, KD, 128]  fp8((w1 - w1a)*2^6), same layout
      w2a [128, KF, D]        w2a[p,kf,d] = fp8(w2[d, kf*128+p])
      w2b [128, KF, D]        fp8((w2 - w2a)*2^6)
    DRAM output:
      yT  [128, KD, C] f32    yT[p,dt,c] = y[c, dt*128+p]
    """
    import concourse.mybir as mybir
    import concourse.tile as tile
    from concourse import bacc

    f8, f32 = mybir.dt.float8e4, mybir.dt.float32
    Relu = mybir.ActivationFunctionType.Relu
    DR = mybir.MatmulPerfMode.DoubleRow
    Alu = mybir.AluOpType

    nc = bacc.Bacc("TRN2", target_bir_lowering=False, debug=False)
    xs0 = nc.dram_tensor("xs0", [P, KD, C], f8, kind="ExternalInput")
    xs1 = nc.dram_tensor("xs1", [P, KD, C], f8, kind="ExternalInput")
    w1a = nc.dram_tensor("w1a", [P, KF, KD, P], f8, kind="ExternalInput")
    w1b = nc.dram_tensor("w1b", [P, KF, KD, P], f8, kind="ExternalInput")
    w2a = nc.dram_tensor("w2a", [P, KF, D_MODEL], f8, kind="ExternalInput")
    w2b = nc.dram_tensor("w2b", [P, KF, D_MODEL], f8, kind="ExternalInput")
    yT = nc.dram_tensor("yT", [P, KD, C], f32, kind="ExternalOutput")

    widths = _block_widths(C)
    blocks = []
    off = 0
    for w in widths:
        blocks.append((off, w))
        off += w

    with tile.TileContext(nc) as tc:
        with (
            tc.tile_pool(name="wpool", bufs=1) as wpool,
            tc.tile_pool(name="xpool", bufs=2) as xpool,
            tc.tile_pool(name="hpool", bufs=1) as hpool,
            tc.tile_pool(name="tpool", bufs=3) as tpool,
            tc.tile_pool(name="ypool", bufs=3) as ypool,
            tc.tile_pool(name="pmp", bufs=2, space="PSUM") as pmp,
            tc.tile_pool(name="pcp", bufs=2, space="PSUM") as pcp,
            tc.tile_pool(name="pymp", bufs=2, space="PSUM") as pymp,
            tc.tile_pool(name="pycp", bufs=2, space="PSUM") as pycp,
        ):
            # ---- SBUF weight tiles (resident) ----
            w1a_ch = [
                wpool.tile([P, WCH, KD, P], f8, tag=f"w1a_{c0}", name=f"w1a_{c0}")
                for c0 in range(0, KF, WCH)
            ]
            w1b_ch = [
                wpool.tile([P, WCH, KD, P], f8, tag=f"w1b_{c0}", name=f"w1b_{c0}")
                for c0 in range(0, KF, WCH)
            ]
            w2a_t = wpool.tile([P, KF, D_MODEL], f8, tag="w2a")
            w2b_t = wpool.tile([P, KF, D_MODEL], f8, tag="w2b")

            # ---- block 0 x tiles + head-of-stream DMAs ----
            xt0 = xpool.tile([P, KD, C_BLK], f8, tag="xt0")
            xt1 = xpool.tile([P, KD, C_BLK], f8, tag="xt1")
            w0 = widths[0]
            # criticality order: first matmul needs w1a[fc0] + all xt0 k-pairs;
            # x1-mains follow 4 instructions later; w1b[fc0] 8 later.
            nc.sync.dma_start(w1a_ch[0][:, 0:1], w1a[:, 0:1])
            for k0 in range(0, KD, 2):
                nc.sync.dma_start(xt0[:, k0 : k0 + 2, :w0], xs0[:, k0 : k0 + 2, 0:w0])
            for k0 in range(0, KD, 2):
                nc.sync.dma_start(xt1[:, k0 : k0 + 2, :w0], xs1[:, k0 : k0 + 2, 0:w0])
            nc.sync.dma_start(w1b_ch[0][:, 0:1], w1b[:, 0:1])
            for j in range(1, WCH):
                nc.sync.dma_start(w1a_ch[0][:, j : j + 1], w1a[:, j : j + 1])
                nc.sync.dma_start(w1b_ch[0][:, j : j + 1], w1b[:, j : j + 1])
            # interleaved weight streams: w1 chunk pairs, with w2 half-D chunks
            # spliced in so L2's first d-tiles are ready when block0 L1 ends
            DH = D_MODEL // 2
            w2_parts = [  # (tile, dram, d0)
                (w2a_t, w2a, 0),
                (w2b_t, w2b, 0),
                (w2a_t, w2a, DH),
                (w2b_t, w2b, DH),
            ]
            w2_sched = {2: 0, 3: 1, 4: 2, 5: 3}  # after w1 chunk-pair i, issue w2 part
            for i, c0 in enumerate(range(WCH, KF, WCH)):
                ci = c0 // WCH
                nc.sync.dma_start(w1a_ch[ci][:], w1a[:, c0 : c0 + WCH])
                nc.sync.dma_start(w1b_ch[ci][:], w1b[:, c0 : c0 + WCH])
                if ci in w2_sched:
                    t, d, d0 = w2_parts[w2_sched[ci]]
                    nc.sync.dma_start(t[:, :, d0 : d0 + DH], d[:, :, d0 : d0 + DH])

            def w1a_ap(fc, kp):
                return w1a_ch[fc // WCH][:, fc % WCH, 2 * kp : 2 * kp + 2, :]

            def w1b_ap(fc, kp):
                return w1b_ch[fc // WCH][:, fc % WCH, 2 * kp : 2 * kp + 2, :]

            for bi, (off, w) in enumerate(blocks):
                if bi > 0:
                    xt0 = xpool.tile([P, KD, C_BLK], f8, tag="xt0")
                    xt1 = xpool.tile([P, KD, C_BLK], f8, tag="xt1")
                    nc.sync.dma_start(xt0[:, :, :w], xs0[:, :, off : off + w])
                    nc.sync.dma_start(xt1[:, :, :w], xs1[:, :, off : off + w])
                h0 = hpool.tile([P, KF, C_BLK], f8, tag="h0")
                h1 = hpool.tile([P, KF, C_BLK], f8, tag="h1")
                # ---- layer 1 ----
                for fc in range(KF):
                    pm = pmp.tile([P, C_BLK], f32, tag="pm")
                    pc = pcp.tile([P, C_BLK], f32, tag="pc")
                    for kp in range(KD // 2):
                        nc.tensor.matmul(
                            pm[:, :w], lhsT=w1a_ap(fc, kp), rhs=xt0[:, 2 * kp : 2 * kp + 2, :w],
                            start=(kp == 0), stop=False, perf_mode=DR,
                        )
                    for kp in range(KD // 2):
                        nc.tensor.matmul(
                            pm[:, :w], lhsT=w1a_ap(fc, kp), rhs=xt1[:, 2 * kp : 2 * kp + 2, :w],
                            start=False, stop=(kp == KD // 2 - 1), perf_mode=DR,
                        )
                    for kp in range(KD // 2):
                        nc.tensor.matmul(
                            pc[:, :w], lhsT=w1b_ap(fc, kp), rhs=xt0[:, 2 * kp : 2 * kp + 2, :w],
                            start=(kp == 0), stop=(kp == KD // 2 - 1), perf_mode=DR,
                        )
                    tp = tpool.tile([P, C_BLK], f32, tag="tp")
                    nc.vector.scalar_tensor_tensor(
                        tp[:, :w], pc[:, :w], 1.0 / SC, pm[:, :w], Alu.mult, Alu.add
                    )
                    nc.scalar.activation(h0[:, fc, :w], tp[:, :w], Relu)
                    nc.vector.scalar_tensor_tensor(
                        h1[:, fc, :w], tp[:, :w], 0.0, h0[:, fc, :w], Alu.max, Alu.subtract
                    )
                # ---- layer 2 ----
                for dt in range(KD):
                    pym = pymp.tile([P, C_BLK], f32, tag="pym")
                    pyc = pycp.tile([P, C_BLK], f32, tag="pyc")
                    dsl = slice(dt * P, (dt + 1) * P)
                    for kp in range(KF // 2):
                        nc.tensor.matmul(
                            pym[:, :w], lhsT=w2a_t[:, 2 * kp : 2 * kp + 2, dsl],
                            rhs=h0[:, 2 * kp : 2 * kp + 2, :w],
                            start=(kp == 0), stop=False, perf_mode=DR,
                        )
                    for kp in range(KF // 2):
                        nc.tensor.matmul(
                            pym[:, :w], lhsT=w2a_t[:, 2 * kp : 2 * kp + 2, dsl],
                            rhs=h1[:, 2 * kp : 2 * kp + 2, :w],
                            start=False, stop=(kp == KF // 2 - 1), perf_mode=DR,
                        )
                    for kp in range(KF // 2):
                        nc.tensor.matmul(
                            pyc[:, :w], lhsT=w2b_t[:, 2 * kp : 2 * kp + 2, dsl],
                            rhs=h0[:, 2 * kp : 2 * kp + 2, :w],
                            start=(kp == 0), stop=(kp == KF // 2 - 1), perf_mode=DR,
                        )
                    yt = ypool.tile([P, C_BLK], f32, tag="yt")
                    nc.vector.scalar_tensor_tensor(
                        yt[:, :w], pyc[:, :w], 1.0 / SC, pym[:, :w], Alu.mult, Alu.add
                    )
                    nc.sync.dma_start(yT[:, dt, off : off + w], yt[:, :w])

    nc.compile()
    return nc


def route_tokens(xf: np.ndarray, gate_w: np.ndarray):
    """Top-2 routing, replicating jax.lax.top_k tie-breaking (lowest index)."""
    logits = xf @ gate_w.astype(np.float32).T  # [T, E]
    top2 = np.argsort(-logits, axis=-1, kind="stable")[:, :TOP_K]
    tv = np.take_along_axis(logits, top2, axis=-1)
    tv = tv - tv.max(axis=-1, keepdims=True)
    ex = np.exp(tv)
    gates = ex / ex.sum(axis=-1, keepdims=True)
    rows, weights = [], []
    for e in range(NUM_EXPERTS):
        r, kpos = np.nonzero(top2 == e)
        rows.append(r)
        weights.append(gates[r, kpos].astype(np.float32))
    return rows, weights


def _fp8_pair(a: np.ndarray, scale: float = 1.0):
    """a ~= a0 + a1/scale with both halves fp8 e4m3."""
    a0 = a.astype(F8)
    a1 = ((a - a0.astype(np.float32)) * scale).astype(F8)
    return a0, a1


def make_expert_inputs(xf, w1, w2, rows, weights, C):
    """Per-core input arrays in the DRAM layouts build_moe_nc expects."""
    in_maps = []
    for e in range(NUM_EXPERTS):
        cnt = len(rows[e])
        Xg = np.zeros((C, D_MODEL), np.float32)
        Xg[:cnt] = xf[rows[e]] * weights[e][:, None]
        X0, X1 = _fp8_pair(Xg)  # unscaled residual: x is O(1)

        def xlay(X):
            return np.ascontiguousarray(X.T.reshape(KD, P, C).transpose(1, 0, 2))

        W1_0, W1_1 = _fp8_pair(w1[e].astype(np.float32), SC)

        def w1lay(W):
            return np.ascontiguousarray(W.reshape(KF, P, KD, P).transpose(3, 0, 2, 1))

        W2_0, W2_1 = _fp8_pair(w2[e].astype(np.float32), SC)

        def w2lay(W):
            return np.ascontiguousarray(W.T.reshape(KF, P, D_MODEL).transpose(1, 0, 2))

        in_maps.append(
            {
                "xs0": xlay(X0),
                "xs1": xlay(X1),
                "w1a": w1lay(W1_0),
                "w1b": w1lay(W1_1),
                "w2a": w2lay(W2_0),
                "w2b": w2lay(W2_1),
            }
        )
    return in_maps


def kernel(x, gate_w, w1, w2):
    from concourse.bass_utils import run_bass_kernel_spmd

    x = np.asarray(x)
    gate_w = np.asarray(gate_w)
    w1 = np.asarray(w1)
    w2 = np.asarray(w2)
    B, S, D = x.shape

    xf = x.reshape(-1, D).astype(np.float32)
    rows, weights = route_tokens(xf, gate_w)
    counts = [len(r) for r in rows]
    C = capacity(max(counts))

    nc = _NC_CACHE.get(C)
    if nc is None:
        nc = _NC_CACHE[C] = build_moe_nc(C)
    in_maps = make_expert_inputs(xf, w1, w2, rows, weights, C)
    res = run_bass_kernel_spmd(nc, in_maps, core_ids=list(range(NUM_EXPERTS)))

    out = np.zeros((B * S, D), np.float32)
    for e in range(NUM_EXPERTS):
        yT = res.results[e]["yT"]  # [P, KD, C]
        y = yT.transpose(2, 1, 0).reshape(C, D_MODEL)
        out[rows[e]] += y[: counts[e]]
    return out.reshape(B, S, D)


# revision 4
# speedup vs baseline: 1.3044x; 1.0104x over previous
"""MoE layer (8 experts, top-2) on 8 Trainium2 NeuronCores.

Expert parallelism with host-side dispatch; fp8 DoubleRow matmuls with full
error compensation, all accumulating in a single fp32 PSUM group per output
tile:
  - Host: gate logits, top-2 + softmax, token->expert dispatch. Gate scales
    are folded into x (relu is positive-homogeneous), so the device computes
    plain y_e = relu(x_e @ w1.T) @ w2.T on pre-scaled tokens.
  - Every operand T is split T = T0 + T1 with both halves fp8 e4m3. The
    residual product rides in the same PSUM at matched scale by pre-scaling
    the weight residual up by 2^4 and the activation main down by 2^4 (both
    exact exponent shifts in fp8):
      psum = x0@w0.T + x1@w0.T + (x0/16)@((w-w0)*16).T
    dropping only the tiny residual*residual term. Each matmul pairs two
    128-deep k-tiles in MatmulPerfMode.DoubleRow (0.5 cycles/row = 4x bf16
    throughput), so the compensated total runs at 1.5x bf16 speed with
    bf16-level accuracy (~3e-3 end to end).
  - Layer 1: h0 = fp8(relu(psum)) (ACT), h1 = fp8(relu(psum)-h0) (DVE stt),
    h0d = h0/16 (ACT). Layer 2 repeats the same 3-group pattern on
    (h0, h1, h0d) against w2 splits, yT written back d-major.
  - Both layers keep tokens on the moving free dim: any block width, no
    128-token padding anywhere (capacity = max expert count, exactly).
  - DMA issue order is hand-scheduled: block0 x + first w1 chunks feed the
    first matmuls within ~2.5us; the w1 chunk stream stays ahead of L1; w2a
    then w2b follow so layer 2's mains/corrections are resident just in time.
"""

import os

os.environ.setdefault("BASS_NEVER_TRACE", "1")

import numpy as np
import ml_dtypes

D_MODEL = 1024
D_FF = 4096
NUM_EXPERTS = 8
TOP_K = 2
P = 128
KD = D_MODEL // P  # 8
KF = D_FF // P  # 32
C_BLK = 512
WCH = 4  # fc chunk size for w1 DMA staging
SC = 16.0  # residual pre-scale (2^4); shifted operands use 1/SC

F8 = ml_dtypes.float8_e4m3

_NC_CACHE: dict[int, object] = {}


def _block_widths(C: int) -> list[int]:
    """First block 512 (buys time for the w2 DMA stream before L2 starts),
    the rest equal-ish. Every block should be >=342 wide so the PE engine
    time per DoubleRow (0.5 cyc/row) stays above the 71 ns sequencer cost."""
    if C <= C_BLK:
        return [C]
    nb = -(-C // C_BLK)
    rest = C - C_BLK
    base = rest // (nb - 1)
    remn = rest % (nb - 1)
    widths = [C_BLK] + [base + (1 if i < remn else 0) for i in range(nb - 1)]
    assert sum(widths) == C
    return widths


def capacity(max_count: int) -> int:
    return max(max_count, 2 * 342)


def build_moe_nc(C: int):
    """Bass/Tile program for one expert shard with token capacity C.

    DRAM inputs (per core), all fp8 e4m3:
      xs0 [128, KD, C]        xs0[p,k,c] = fp8(g_c * x_c)[k*128+p]
      xs1 [128, KD, C]        fp8 residual of the above
      xsd [128, KD, C]        fp8(xs0 / 16) (exact shift)
      w1a [128, KF, KD, 128]  w1a[p,fc,k,j] = fp8(w1[fc*128+j, k*128+p])
      w1b [128, KF, KD, 128]  fp8((w1 - w1a)*16), same layout
      w2a [128, KF, D]        w2a[p,kf,d] = fp8(w2[d, kf*128+p])
      w2b [128, KF, D]        fp8((w2 - w2a)*16)
    DRAM output:
      yT  [128, KD, C] f32    yT[p,dt,c] = y[c, dt*128+p]
    """
    import concourse.mybir as mybir
    import concourse.tile as tile
    from concourse import bacc

    f8, f32 = mybir.dt.float8e4, mybir.dt.float32
    Relu = mybir.ActivationFunctionType.Relu
    DR = mybir.MatmulPerfMode.DoubleRow
    Alu = mybir.AluOpType

    nc = bacc.Bacc("TRN2", target_bir_lowering=False, debug=False)
    xs0 = nc.dram_tensor("xs0", [P, KD, C], f8, kind="ExternalInput")
    xs1 = nc.dram_tensor("xs1", [P, KD, C], f8, kind="ExternalInput")
    xsd = nc.dram_tensor("xsd", [P, KD, C], f8, kind="ExternalInput")
    w1a = nc.dram_tensor("w1a", [P, KF, KD, P], f8, kind="ExternalInput")
    w1b = nc.dram_tensor("w1b", [P, KF, KD, P], f8, kind="ExternalInput")
    w2a = nc.dram_tensor("w2a", [P, KF, D_MODEL], f8, kind="ExternalInput")
    w2b = nc.dram_tensor("w2b", [P, KF, D_MODEL], f8, kind="ExternalInput")
    yT = nc.dram_tensor("yT", [P, KD, C], f32, kind="ExternalOutput")

    widths = _block_widths(C)
    blocks = []
    off = 0
    for w in widths:
        blocks.append((off, w))
        off += w

    with tile.TileContext(nc) as tc:
        with (
            tc.tile_pool(name="wpool", bufs=1) as wpool,
            tc.tile_pool(name="xpool", bufs=2) as xpool,
            tc.tile_pool(name="hpool", bufs=1) as hpool,
            tc.tile_pool(name="ypool", bufs=3) as ypool,
            tc.tile_pool(name="pmp", bufs=4, space="PSUM") as pmp,
            tc.tile_pool(name="pymp", bufs=3, space="PSUM") as pymp,
        ):
            # ---- SBUF weight tiles (resident) ----
            w1a_ch = [
                wpool.tile([P, WCH, KD, P], f8, tag=f"w1a_{c0}", name=f"w1a_{c0}")
                for c0 in range(0, KF, WCH)
            ]
            w1b_ch = [
                wpool.tile([P, WCH, KD, P], f8, tag=f"w1b_{c0}", name=f"w1b_{c0}")
                for c0 in range(0, KF, WCH)
            ]
            w2a_t = wpool.tile([P, KF, D_MODEL], f8, tag="w2a", name="w2a_t")
            w2b_t = wpool.tile([P, KF, D_MODEL], f8, tag="w2b", name="w2b_t")

            # ---- block 0 x tiles + head-of-stream DMAs (criticality order) ----
            xt0 = xpool.tile([P, KD, C_BLK], f8, tag="xt0", name="xt0_0")
            xt1 = xpool.tile([P, KD, C_BLK], f8, tag="xt1", name="xt1_0")
            xtd = xpool.tile([P, KD, C_BLK], f8, tag="xtd", name="xtd_0")
            w0 = widths[0]
            nc.sync.dma_start(w1a_ch[0][:, 0:1], w1a[:, 0:1])
            for k0 in range(0, KD, 2):
                nc.sync.dma_start(xt0[:, k0 : k0 + 2, :w0], xs0[:, k0 : k0 + 2, 0:w0])
            for k0 in range(0, KD, 2):
                nc.sync.dma_start(xt1[:, k0 : k0 + 2, :w0], xs1[:, k0 : k0 + 2, 0:w0])
            nc.sync.dma_start(w1b_ch[0][:, 0:1], w1b[:, 0:1])
            for k0 in range(0, KD, 2):
                nc.sync.dma_start(xtd[:, k0 : k0 + 2, :w0], xsd[:, k0 : k0 + 2, 0:w0])
            for j in range(1, WCH):
                nc.sync.dma_start(w1a_ch[0][:, j : j + 1], w1a[:, j : j + 1])
                nc.sync.dma_start(w1b_ch[0][:, j : j + 1], w1b[:, j : j + 1])
            # w1 chunk stream (stays well ahead of L1 consumption), then w2a
            # halves (layer-2 mains), then w2b halves (layer-2 corrections)
            for c0 in range(WCH, KF, WCH):
                ci = c0 // WCH
                nc.sync.dma_start(w1a_ch[ci][:], w1a[:, c0 : c0 + WCH])
                nc.sync.dma_start(w1b_ch[ci][:], w1b[:, c0 : c0 + WCH])
            DH = D_MODEL // 2
            for d0 in (0, DH):
                nc.sync.dma_start(w2a_t[:, :, d0 : d0 + DH], w2a[:, :, d0 : d0 + DH])
            for d0 in (0, DH):
                nc.sync.dma_start(w2b_t[:, :, d0 : d0 + DH], w2b[:, :, d0 : d0 + DH])

            def w1a_ap(fc, kp):
                return w1a_ch[fc // WCH][:, fc % WCH, 2 * kp : 2 * kp + 2, :]

            def w1b_ap(fc, kp):
                return w1b_ch[fc // WCH][:, fc % WCH, 2 * kp : 2 * kp + 2, :]

            for bi, (off, w) in enumerate(blocks):
                if bi > 0:
                    xt0 = xpool.tile([P, KD, C_BLK], f8, tag="xt0", name=f"xt0_{bi}")
                    xt1 = xpool.tile([P, KD, C_BLK], f8, tag="xt1", name=f"xt1_{bi}")
                    xtd = xpool.tile([P, KD, C_BLK], f8, tag="xtd", name=f"xtd_{bi}")
                    nc.sync.dma_start(xt0[:, :, :w], xs0[:, :, off : off + w])
                    nc.sync.dma_start(xt1[:, :, :w], xs1[:, :, off : off + w])
                    nc.sync.dma_start(xtd[:, :, :w], xsd[:, :, off : off + w])
                h0 = hpool.tile([P, KF, C_BLK], f8, tag="h0", name=f"h0_{bi}")
                h1 = hpool.tile([P, KF, C_BLK], f8, tag="h1", name=f"h1_{bi}")
                h0d = hpool.tile([P, KF, C_BLK], f8, tag="h0d", name=f"h0d_{bi}")
                # ---- layer 1: 12 DoubleRows into one PSUM per f-tile ----
                for fc in range(KF):
                    pm = pmp.tile([P, C_BLK], f32, tag="pm", name=f"pm_{bi}_{fc}")
                    for kp in range(KD // 2):
                        nc.tensor.matmul(
                            pm[:, :w], lhsT=w1a_ap(fc, kp),
                            rhs=xt0[:, 2 * kp : 2 * kp + 2, :w],
                            start=(kp == 0), stop=False, perf_mode=DR,
                        )
                    for kp in range(KD // 2):
                        nc.tensor.matmul(
                            pm[:, :w], lhsT=w1a_ap(fc, kp),
                            rhs=xt1[:, 2 * kp : 2 * kp + 2, :w],
                            start=False, stop=False, perf_mode=DR,
                        )
                    for kp in range(KD // 2):
                        nc.tensor.matmul(
                            pm[:, :w], lhsT=w1b_ap(fc, kp),
                            rhs=xtd[:, 2 * kp : 2 * kp + 2, :w],
                            start=False, stop=(kp == KD // 2 - 1), perf_mode=DR,
                        )
                    nc.scalar.activation(h0[:, fc, :w], pm[:, :w], Relu)
                    nc.vector.scalar_tensor_tensor(
                        h1[:, fc, :w], pm[:, :w], 0.0, h0[:, fc, :w],
                        Alu.max, Alu.subtract,
                    )
                    nc.scalar.mul(h0d[:, fc, :w], h0[:, fc, :w], 1.0 / SC)
                # ---- layer 2: 48 DoubleRows into one PSUM per d-tile ----
                for dt in range(KD):
                    pym = pymp.tile([P, C_BLK], f32, tag="pym", name=f"pym_{bi}_{dt}")
                    dsl = slice(dt * P, (dt + 1) * P)
                    for kp in range(KF // 2):
                        nc.tensor.matmul(
                            pym[:, :w], lhsT=w2a_t[:, 2 * kp : 2 * kp + 2, dsl],
                            rhs=h0[:, 2 * kp : 2 * kp + 2, :w],
                            start=(kp == 0), stop=False, perf_mode=DR,
                        )
                    for kp in range(KF // 2):
                        nc.tensor.matmul(
                            pym[:, :w], lhsT=w2a_t[:, 2 * kp : 2 * kp + 2, dsl],
                            rhs=h1[:, 2 * kp : 2 * kp + 2, :w],
                            start=False, stop=False, perf_mode=DR,
                        )
                    for kp in range(KF // 2):
                        nc.tensor.matmul(
                            pym[:, :w], lhsT=w2b_t[:, 2 * kp : 2 * kp + 2, dsl],
                            rhs=h0d[:, 2 * kp : 2 * kp + 2, :w],
                            start=False, stop=(kp == KF // 2 - 1), perf_mode=DR,
                        )
                    yt = ypool.tile([P, C_BLK], f32, tag="yt", name=f"yt_{bi}_{dt}")
                    nc.scalar.copy(yt[:, :w], pym[:, :w])
                    nc.sync.dma_start(yT[:, dt, off : off + w], yt[:, :w])

    nc.compile()
    return nc


def route_tokens(xf: np.ndarray, gate_w: np.ndarray):
    """Top-2 routing, replicating jax.lax.top_k tie-breaking (lowest index)."""
    logits = xf @ gate_w.astype(np.float32).T  # [T, E]
    top2 = np.argsort(-logits, axis=-1, kind="stable")[:, :TOP_K]
    tv = np.take_along_axis(logits, top2, axis=-1)
    tv = tv - tv.max(axis=-1, keepdims=True)
    ex = np.exp(tv)
    gates = ex / ex.sum(axis=-1, keepdims=True)
    rows, weights = [], []
    for e in range(NUM_EXPERTS):
        r, kpos = np.nonzero(top2 == e)
        rows.append(r)
        weights.append(gates[r, kpos].astype(np.float32))
    return rows, weights


def _fp8_pair(a: np.ndarray, scale: float = 1.0):
    """a ~= a0 + a1/scale with both halves fp8 e4m3."""
    a0 = a.astype(F8)
    a1 = ((a - a0.astype(np.float32)) * scale).astype(F8)
    return a0, a1


def make_expert_inputs(xf, w1, w2, rows, weights, C):
    """Per-core input arrays in the DRAM layouts build_moe_nc expects."""
    in_maps = []
    for e in range(NUM_EXPERTS):
        cnt = len(rows[e])
        Xg = np.zeros((C, D_MODEL), np.float32)
        Xg[:cnt] = xf[rows[e]] * weights[e][:, None]
        X0, X1 = _fp8_pair(Xg)
        Xd = (X0.astype(np.float32) / SC).astype(F8)

        def xlay(X):
            return np.ascontiguousarray(X.T.reshape(KD, P, C).transpose(1, 0, 2))

        W1_0, W1_1 = _fp8_pair(w1[e].astype(np.float32), SC)

        def w1lay(W):
            return np.ascontiguousarray(W.reshape(KF, P, KD, P).transpose(3, 0, 2, 1))

        W2_0, W2_1 = _fp8_pair(w2[e].astype(np.float32), SC)

        def w2lay(W):
            return np.ascontiguousarray(W.T.reshape(KF, P, D_MODEL).transpose(1, 0, 2))

        in_maps.append(
            {
                "xs0": xlay(X0),
                "xs1": xlay(X1),
                "xsd": xlay(Xd),
                "w1a": w1lay(W1_0),
                "w1b": w1lay(W1_1),
                "w2a": w2lay(W2_0),
                "w2b": w2lay(W2_1),
            }
        )
    return in_maps


def kernel(x, gate_w, w1, w2):
    from concourse.bass_utils import run_bass_kernel_spmd

    x = np.asarray(x)
    gate_w = np.asarray(gate_w)
    w1 = np.asarray(w1)
    w2 = np.asarray(w2)
    B, S, D = x.shape

    xf = x.reshape(-1, D).astype(np.float32)
    rows, weights = route_tokens(xf, gate_w)
    counts = [len(r) for r in rows]
    C = capacity(max(counts))

    nc = _NC_CACHE.get(C)
    if nc is None:
        nc = _NC_CACHE[C] = build_moe_nc(C)
    in_maps = make_expert_inputs(xf, w1, w2, rows, weights, C)
    res = run_bass_kernel_spmd(nc, in_maps, core_ids=list(range(NUM_EXPERTS)))

    out = np.zeros((B * S, D), np.float32)
    for e in range(NUM_EXPERTS):
        yT = res.results[e]["yT"]  # [P, KD, C]
        y = yT.transpose(2, 1, 0).reshape(C, D_MODEL)
        out[rows[e]] += y[: counts[e]]
    return out.reshape(B, S, D)
